# revision 42
# baseline (speedup 1.0000x reference)
"""Multi-head attention (B=2, S=2048, D=1024, H=16) on 8 trn2 NeuronCores.

Sharding: 2 groups of 4 cores; group b owns batch b, core (group rank r)
owns heads [4r:4r+4] (channels [256r:256r+256]). Each core loads only its
batch's x (8.4 MB), transposes it via the PE, projects q/k/v for its 4
heads, and runs attention. The output projection is re-sharded by sequence
rows: destination core d takes rows [256d:256d+256) of BOTH batches.

vs the 375us v1 baseline:
- x is PE-transposed in f32r (1.5 cyc/row); the PSUM->SBUF copy converts
  to bf16 so every downstream matmul runs uniform bf16 (weights/q/k/v/et/
  wo/h) at 1 col/cycle with FWL weight loads.  (Bulk DVE f32->bf16 casts
  measured ~5x below spec; mixed f32r x bf16 matmuls are rejected.)
- The AllToAll is split into 4 chunks overlapped with stage-B compute.
  qT is stored stripe-permuted (column 512g+64d+k holds token 256d+64g+k)
  so attention i-chunk g covers stripe g of EVERY destination; its A2A
  chunk fires while chunk g+1 computes.  Out-proj block 0 interleaves
  into the PE stream mid stage B; block 1's stripe-2 half computes inside
  the last A2A's ~20us latency window, leaving only its stripe-3 half +
  y writes serial (was ~67us of dead tail in v1).
- Per-head score tiles (one PSUM bank each, 4 bufs) + 4 separate attn@v
  accumulator banks: no pair-transition PE stalls (which also re-trigger
  the HAM 4/8 clock throttle).
- Softmax denominators come free from the [v|ones|v] attn@v trick; the
  per-pair normalize is 2 partition-shift copies + ONE exact DVE
  reciprocal (column-bound ~4us/512 cols) + 2 multiplies.
  reciprocal_approx_fast (custom-DVE table) computes garbage under this
  runtime; Ln/Exp on scalar thrashes ACT tables (~2.6us per switch).
- DMA: x split across the sync AND scalar HWDGE queues (~140GB/s each);
  gpsimd SWDGE carries biases -> weights -> bo in need-order (a 4KB bo
  load queued behind the weights once stalled the whole PE for 14us);
  warm-up transposes run during the first x-chunk wait (HAM clock gate
  needs ~3.4us of sustained PE activity to release full clock).

Matmuls keep a uniform K=128 stream: per-head d_k=64 scores contraction is
zero-padded to 128 (kT stored per-head with the sibling head's 64 rows
zeroed); mixing K=64/K=128 measurably degrades every matmul (~724ns vs
~227ns).
"""

import numpy as np

import concourse.bass as bass
import concourse.mybir as mybir
import concourse.tile as tile
from concourse import bacc
from concourse.masks import make_identity
from concourse.bass_utils import run_bass_kernel_spmd

# problem constants (hardcoded per harness contract)
B, S, D = 2, 2048, 1024
H, DK = 16, 64
NCORES = 8
GPB = NCORES // B          # cores per batch group = 4
HPC = H // NCORES * B      # heads per core = 4
NPAIR = HPC // 2           # head pairs per core = 2
CS = HPC * DK              # per-core channel slice = 256
TCH = 512                  # stage-A t-chunk
NTCH = S // TCH            # 4 chunks (one batch per core)
IB = 512                   # stage-B i-chunk
RB = 256                   # output rows per (core, batch)
NSTRIPE = 4                # A2A chunks == stage-B i-chunks
SW = RB // NSTRIPE         # stripe width = 64 rows per dest per chunk
P = 128
F32 = mybir.dt.float32
F32R = mybir.dt.float32r
BF16 = mybir.dt.bfloat16
AF = mybir.ActivationFunctionType
ALU = mybir.AluOpType


def build_nc():
    nc = bacc.Bacc("TRN2", target_bir_lowering=False, debug=False, num_devices=NCORES)

    x = nc.dram_tensor("x", [S, D], F32R, kind="ExternalInput")
    wqT = nc.dram_tensor("wqT", [D, CS], BF16, kind="ExternalInput")
    wkT = nc.dram_tensor("wkT", [D, CS], BF16, kind="ExternalInput")
    wvT = nc.dram_tensor("wvT", [D, CS], BF16, kind="ExternalInput")
    woT = nc.dram_tensor("woT", [D, D], BF16, kind="ExternalInput")
    bq = nc.dram_tensor("bq", [P, NPAIR], F32, kind="ExternalInput")
    bk = nc.dram_tensor("bk", [P, NPAIR], F32, kind="ExternalInput")
    bv = nc.dram_tensor("bv", [P, NPAIR], F32, kind="ExternalInput")
    bo = nc.dram_tensor("bo", [1, D], F32R, kind="ExternalInput")
    y = nc.dram_tensor("y", [2 * RB, D], F32, kind="ExternalOutput")

    with tile.TileContext(nc) as tc:
        with (
            tc.tile_pool(name="const", bufs=1) as cpool,
            tc.tile_pool(name="persist", bufs=1) as ppool,
            tc.tile_pool(name="dram", bufs=1, space="DRAM") as dpool,
        ):
            # identity first: the warm-up transposes and first x transposes
            # gate on it
            ident32 = cpool.tile([P, P], F32)
            make_identity(nc, ident32[:])
            identR_t = cpool.tile([P, P], F32R)
            nc.vector.tensor_copy(identR_t[:], ident32[:])
            identR = identR_t[:]
            identB = cpool.tile([P, P], BF16)
            nc.vector.tensor_copy(identB[:], ident32[:])

            # gpsimd SWDGE queue order = need order: small biases, then the
            # 1.5MB of weights (needed ~25us in), bo last (needed at stage C).
            # The queue is FIFO and moves only ~125GB/s.
            bq_sb = cpool.tile([P, NPAIR], F32)
            bk_sb = cpool.tile([P, NPAIR], F32)
            bv_sb = cpool.tile([P, NPAIR], F32)
            nc.gpsimd.dma_start(bq_sb[:], bq[:])
            nc.gpsimd.dma_start(bk_sb[:], bk[:])
            nc.gpsimd.dma_start(bv_sb[:], bv[:])
            wq_sb = cpool.tile([P, 8, CS], BF16)
            wk_sb = cpool.tile([P, 8, CS], BF16)
            wv_sb = cpool.tile([P, 8, CS], BF16)
            nc.gpsimd.dma_start(wq_sb[:], wqT[:].rearrange("(o p) c -> p o c", p=P))
            nc.gpsimd.dma_start(wk_sb[:], wkT[:].rearrange("(o p) c -> p o c", p=P))
            nc.gpsimd.dma_start(wv_sb[:], wvT[:].rearrange("(o p) c -> p o c", p=P))
            bo_sb = cpool.tile([1, D], F32R)
            nc.gpsimd.dma_start(bo_sb[:], bo[:])

            ones1 = cpool.tile([1, P], F32)
            nc.vector.memset(ones1[:], 1.0)
            ones_row = cpool.tile([1, P], F32R)
            nc.vector.tensor_copy(ones_row[:], ones1[:])
            bo_full = cpool.tile([P, D], F32)

            # persistent activations.  qT is stripe-permuted: [P, g, d, k]
            # where column (g, d, k) holds token 256d + 64g + k.
            qTq = [ppool.tile([P, NSTRIPE, NCORES, SW], BF16, name=f"qT{pp_}")
                   for pp_ in range(NPAIR)]
            kTh = [ppool.tile([P, S], BF16, name=f"kTh{h}") for h in range(HPC)]
            for h in range(HPC):
                z0 = (1 - h % 2) * DK  # zero rows: even head -> 64:128, odd -> 0:64
                nc.vector.memset(kTh[h][z0:z0 + DK, :], 0.0)
            # v per head-pair j-tile stored [v_even | ones | v_odd] (192 cols):
            # head h's attn@v lhsT is the contiguous 128-col slice at 64*(h%2);
            # the shared ones block replicates the softmax denominator onto the
            # other 64 output partitions.
            v_sbp = [ppool.tile([P, S // P, 3 * DK], BF16, name=f"v{pp_}")
                     for pp_ in range(NPAIR)]
            for pp_ in range(NPAIR):
                nc.vector.memset(v_sbp[pp_][:, :, DK:2 * DK], 1.0)

            # PE warm-up during the first x-chunk DMA wait: the HAM clock
            # gate needs ~3.4us of sustained activity to release 2.4 GHz.
            with tc.tile_pool(name="psW", bufs=2, space="PSUM") as psW:
                for w in range(6):
                    pw = psW.tile([P, 4 * P], BF16, tag="warm")
                    for t in range(4):
                        nc.tensor.transpose(
                            pw[:, t * P:(t + 1) * P], identB[:], identB[:])

            # ---- stage A: x cast+transpose + q/k/v projections ----
            with (
                tc.tile_pool(name="aw", bufs=1) as awpool,
                tc.tile_pool(name="vtc", bufs=2) as vtcpool,
                tc.tile_pool(name="stageA", bufs=2) as apool,
                tc.tile_pool(name="psA", bufs=2, space="PSUM") as psA,
                tc.tile_pool(name="psP", bufs=3, space="PSUM") as psP,
            ):
                for te in range(NTCH):
                    x_e = apool.tile([P, TCH // P, D], F32R, tag="x_e", bufs=3)
                    # split every chunk across the sync AND scalar HWDGE
                    # queues — one queue sustains only ~140GB/s, two nearly
                    # double the x feed rate
                    if te == 0:
                        # split the first load finer so transposes start sooner
                        for hh in range(4):
                            r0 = hh * (TCH // 4)
                            for qq in range(2):
                                eng = nc.sync if qq == 0 else nc.scalar
                                eng.dma_start(
                                    x_e[:, hh:hh + 1, qq * 512:(qq + 1) * 512],
                                    x[r0:r0 + TCH // 4,
                                      qq * 512:(qq + 1) * 512].rearrange(
                                        "(tt p) d -> p tt d", p=P
                                    ),
                                )
                    else:
                        for half in range(2):
                            eng = nc.sync if half == 0 else nc.scalar
                            eng.dma_start(
                                x_e[:, 2 * half:2 * half + 2, :],
                                x[te * TCH + half * (TCH // 2):
                                  te * TCH + (half + 1) * (TCH // 2), :
                                  ].rearrange("(tt p) d -> p tt d", p=P),
                            )
                    # f32r PE transposes (1.5 cyc/row); the PSUM->SBUF copy
                    # does the bf16 conversion so the projection matmuls get
                    # uniform bf16 operands (mixed f32r/bf16 is rejected, and
                    # a separate bulk DVE cast measured ~5x slower than spec)
                    xT_e = apool.tile([P, 8, TCH], BF16, tag="xT_e")
                    for dd in range(8):
                        ps = psA.tile([P, TCH], F32R, tag="trps")
                        for tt in range(TCH // P):
                            nc.tensor.transpose(
                                ps[:, tt * P:(tt + 1) * P],
                                x_e[:, tt, dd * P:(dd + 1) * P],
                                identR,
                            )
                        if dd % 2 == 0:
                            nc.vector.tensor_copy(xT_e[:, dd, :], ps[:])
                        else:
                            nc.scalar.activation(
                                xT_e[:, dd, :], ps[:], AF.Copy)
                    sl = slice(te * TCH, (te + 1) * TCH)
                    vTc = [None, None]
                    for proj, (w_sb, b_sb) in enumerate(
                        ((wq_sb, bq_sb), (wk_sb, bk_sb), (wv_sb, bv_sb))
                    ):
                        for ct in range(NPAIR):
                            cs = slice(ct * P, (ct + 1) * P)
                            pp = psP.tile([P, TCH], F32, tag="projps")
                            for dd in range(8):
                                nc.tensor.matmul(
                                    pp[:],
                                    w_sb[:, dd, cs],
                                    xT_e[:, dd, :],
                                    start=(dd == 0),
                                    stop=(dd == 7),
                                )
                            if proj == 0:
                                # stripe-permuted write: token 256dl+64g+k of
                                # this chunk -> qT[:, g, 2te+dl, k]
                                for dl in range(2):
                                    nc.vector.tensor_tensor(
                                        qTq[ct][:, :, 2 * te + dl, :],
                                        pp[:, dl * 256:(dl + 1) * 256].rearrange(
                                            "p (g k) -> p g k", g=NSTRIPE
                                        ),
                                        b_sb[:, ct:ct + 1, None].to_broadcast(
                                            [P, NSTRIPE, SW]),
                                        ALU.add,
                                    )
                            elif proj == 1:
                                for hh in range(2):
                                    hs = slice(hh * DK, (hh + 1) * DK)
                                    nc.vector.tensor_tensor(
                                        kTh[2 * ct + hh][hs, sl],
                                        pp[hs, :],
                                        b_sb[hs, ct:ct + 1
                                             ].to_broadcast([DK, TCH]),
                                        ALU.add,
                                    )
                            else:
                                vt = vtcpool.tile([P, TCH], BF16,
                                                  tag=f"vTc{ct}", name=f"vTc{ct}")
                                nc.vector.tensor_tensor(
                                    vt[:], pp[:],
                                    b_sb[:, ct:ct + 1].to_broadcast([P, TCH]),
                                    ALU.add,
                                )
                                vTc[ct] = vt
                    # vT [c, t] -> v natural [t, c] split around the ones block
                    for pp_ in range(NPAIR):
                        for tt in range(TCH // P):
                            psv = psA.tile([P, P], BF16, tag="vtr")
                            nc.tensor.transpose(
                                psv[:], vTc[pp_][:, tt * P:(tt + 1) * P],
                                identB[:])
                            ti = te * (TCH // P) + tt
                            # plain Copy on scalar is fast (unlike its
                            # bias-add path); keeps DVE for the bias-adds
                            nc.scalar.activation(
                                v_sbp[pp_][:, ti, 0:DK], psv[:, 0:DK],
                                AF.Copy)
                            nc.scalar.activation(
                                v_sbp[pp_][:, ti, 2 * DK:3 * DK],
                                psv[:, DK:2 * DK], AF.Copy)

            # ---- stage B (attention) with chunked A2A + interleaved
            # stage C (output projection) ----
            with (
                tc.tile_pool(name="stageC", bufs=1) as c2pool,
                tc.tile_pool(name="yout", bufs=2) as ypool,
            ):
                # the 2 MB wo load overlaps stage-B compute on the sync queue
                wo_sb = c2pool.tile([P, 8, D], BF16)
                nc.sync.dma_start(wo_sb[:], woT[:].rearrange("(o p) n -> p o n", p=P))
                h_sb = c2pool.tile([P, 2, 8, 2 * P], BF16)

                # bo broadcast to all partitions via two K=1 matmuls, emitted
                # here (bo isn't needed until stage C; emitting it at the top
                # made the whole PE queue wait ~27us for the bo DMA)
                with tc.tile_pool(name="psbo", bufs=1, space="PSUM") as psbo:
                    for nch in range(D // 512):
                        pb = psbo.tile([P, 512], F32, tag="pb")
                        nc.tensor.matmul(
                            pb[:], ones_row[:],
                            bo_sb[:, nch * 512:(nch + 1) * 512],
                            start=True, stop=True,
                        )
                        nc.vector.tensor_copy(
                            bo_full[:, nch * 512:(nch + 1) * 512], pb[:])

                a2a_in = [dpool.tile([NCORES, CS, SW], BF16, name=f"a2ai{g}")
                          for g in range(NSTRIPE)]
                a2a_out = [dpool.tile([NCORES, CS, SW], BF16, name=f"a2ao{g}")
                           for g in range(NSTRIPE)]

                with (
                    tc.tile_pool(name="et", bufs=6) as etpool,
                    tc.tile_pool(name="ob", bufs=2) as obpool,
                    tc.tile_pool(name="psS", bufs=4, space="PSUM") as psS,
                    tc.tile_pool(name="psAV", bufs=1, space="PSUM") as psAV,
                ):
                    def load_h_stripe(g):
                        # pull A2A chunk g's payload into h_sb columns
                        # [64g, 64g+64) — emitted once chunk g's collective
                        # is safely complete so the sync queue never blocks
                        # (except stripe 3, which is the tail: split it
                        # across both HWDGE queues)
                        it, st = g // 2, g % 2
                        c0 = it * P + st * SW
                        for hb in range(2):
                            eng = nc.scalar if (g == 3 and hb == 1) else nc.sync
                            eng.dma_start(
                                h_sb[:, hb, :, c0:c0 + SW],
                                a2a_out[g][
                                    hb * GPB:(hb + 1) * GPB, :, :
                                ].rearrange("s (o2 p) k -> p (s o2) k", p=P),
                            )

                    def stage_c_block(it):
                        # out-proj for my rows [128it:128it+128) of each batch
                        for hb in range(2):
                            y_sb = ypool.tile([P, D], F32, tag="y")
                            for nch in range(2):
                                ns = slice(nch * 512, (nch + 1) * 512)
                                py = psS.tile([P, IB], F32, tag="s", name="py")
                                for o in range(8):
                                    nc.tensor.matmul(
                                        py[:],
                                        h_sb[:, hb, o, it * P:(it + 1) * P],
                                        wo_sb[:, o, ns],
                                        start=(o == 0),
                                        stop=(o == 7),
                                    )
                                nc.vector.tensor_tensor(
                                    y_sb[:, ns], py[:], bo_full[:, ns], ALU.add)
                            r0 = hb * RB + it * P
                            nc.sync.dma_start(y[r0:r0 + P, :], y_sb[:])

                    for g in range(NSTRIPE):
                        for pp_ in range(NPAIR):
                            # 4 distinct accumulator banks: no pair-transition
                            # stall waiting for the previous pair's normalize
                            av_e = psAV.tile([P, IB], F32, tag=f"av{2 * pp_}",
                                             name=f"av{2 * pp_}")
                            av_o = psAV.tile([P, IB], F32, tag=f"av{2 * pp_ + 1}",
                                             name=f"av{2 * pp_ + 1}")
                            qTg = qTq[pp_][:, g].rearrange("p d k -> p (d k)")
                            for jt in range(S // P):
                                j0 = jt * P
                                et = etpool.tile([P, 2 * IB], BF16, tag="et")
                                for hh in range(2):
                                    sps = psS.tile([P, IB], F32, tag="s",
                                                   name=f"sps{hh}")
                                    nc.tensor.matmul(
                                        sps[:],
                                        kTh[2 * pp_ + hh][:, j0:j0 + P],
                                        qTg,
                                        start=True,
                                        stop=True,
                                    )
                                    nc.scalar.activation(
                                        et[:, hh * IB:(hh + 1) * IB],
                                        sps[:], AF.Exp, scale=0.125)
                                for hh, av in ((0, av_e), (1, av_o)):
                                    nc.tensor.matmul(
                                        av[:],
                                        v_sbp[pp_][:, jt, hh * DK:hh * DK + 2 * DK],
                                        et[:, hh * IB:(hh + 1) * IB],
                                        start=(jt == 0),
                                        stop=(jt == S // P - 1),
                                    )
                            # pair-merged normalize: even head raw rows 0:64 /
                            # denom 64:128; odd head flipped.  The 4 av banks
                            # keep this whole chain off the PE critical path.
                            # (Ln/Exp-on-scalar thrashes ACT tables ~2.6us per
                            # use; reciprocal_approx_fast returns wrong
                            # results under this runtime; DVE reciprocal is
                            # column-bound so merge both heads into ONE call.)
                            den = obpool.tile([P, IB], F32, tag="den")
                            nc.vector.tensor_copy(den[0:DK, :], av_e[DK:P, :])
                            nc.vector.tensor_copy(den[DK:P, :], av_o[0:DK, :])
                            rec = obpool.tile([P, IB], F32, tag="rec")
                            nc.vector.reciprocal(rec[:], den[:])
                            onrm = obpool.tile([P, IB], BF16, tag="onrm")
                            nc.vector.tensor_tensor(
                                onrm[0:DK, :], av_e[0:DK, :], rec[0:DK, :],
                                ALU.mult)
                            nc.vector.tensor_tensor(
                                onrm[DK:P, :], av_o[DK:P, :], rec[DK:P, :],
                                ALU.mult)
                            # chunk-local column (d, k) -> dest core d, row k
                            nc.sync.dma_start(
                                a2a_in[g][:, pp_ * P:(pp_ + 1) * P, :
                                          ].rearrange("d p k -> p d k"),
                                onrm[:].rearrange("p (d k) -> p d k", d=NCORES),
                            )
                        nc.gpsimd.collective_compute(
                            "AllToAll",
                            ALU.bypass,
                            replica_groups=[list(range(NCORES))],
                            ins=[a2a_in[g].opt()],
                            outs=[a2a_out[g].opt()],
                        )
                        # chunk g-1's collective finished while chunk g
                        # computed: its h-load won't block the sync queue
                        if g >= 1:
                            load_h_stripe(g - 1)
                        if g == 2:
                            # chunks 0+1 landed while chunk 2 computed
                            stage_c_block(0)
                    # ---- tail: split block 1's out-proj so the stripe-2
                    # half (arrived with A2A#2) computes DURING the A2A#3
                    # wait; only the stripe-3 half + y writes remain after ----
                    it = 1
                    pys = {}
                    for hb in range(2):
                        for nch in range(2):
                            ns = slice(nch * 512, (nch + 1) * 512)
                            py = psS.tile([P, IB], F32, tag="s",
                                          name=f"py{hb}{nch}")
                            pys[hb, nch] = py
                            for o in range(8):
                                nc.tensor.matmul(
                                    py[0:SW, :],
                                    h_sb[:, hb, o, it * P:it * P + SW],
                                    wo_sb[:, o, ns],
                                    start=(o == 0),
                                    stop=(o == 7),
                                )
                    load_h_stripe(3)
                    for hb in range(2):
                        y_sb = ypool.tile([P, D], F32, tag="y")
                        for nch in range(2):
                            ns = slice(nch * 512, (nch + 1) * 512)
                            py = pys[hb, nch]
                            for o in range(8):
                                nc.tensor.matmul(
                                    py[SW:P, :],
                                    h_sb[:, hb, o, it * P + SW:(it + 1) * P],
                                    wo_sb[:, o, ns],
                                    start=(o == 0),
                                    stop=(o == 7),
                                )
                            nc.vector.tensor_tensor(
                                y_sb[:, ns], py[:], bo_full[:, ns], ALU.add)
                        r0 = hb * RB + it * P
                        eng = nc.scalar if hb == 1 else nc.sync
                        eng.dma_start(y[r0:r0 + P, :], y_sb[:])

    nc.compile()
    return nc


_NC = None


def _get_nc():
    global _NC
    if _NC is None:
        _NC = build_nc()
    return _NC


def _make_in_maps(x, Wq, bq, Wk, bk, Wv, bv, Wo, bo):
    import ml_dtypes
    xf = np.asarray(x, np.float32).reshape(B, S, D)
    woT = np.ascontiguousarray(np.asarray(Wo, np.float32).T).astype(ml_dtypes.bfloat16)
    bo_r = np.ascontiguousarray(np.asarray(bo, np.float32).reshape(1, D))
    Wq = np.asarray(Wq, np.float32)
    Wk = np.asarray(Wk, np.float32)
    Wv = np.asarray(Wv, np.float32)
    in_maps = []
    for c in range(NCORES):
        b = c // GPB
        r = c % GPB
        sl = slice(r * CS, (r + 1) * CS)
        in_maps.append({
            "x": np.ascontiguousarray(xf[b]),
            "wqT": np.ascontiguousarray(Wq[sl, :].T).astype(ml_dtypes.bfloat16),
            "wkT": np.ascontiguousarray(Wk[sl, :].T).astype(ml_dtypes.bfloat16),
            "wvT": np.ascontiguousarray(Wv[sl, :].T).astype(ml_dtypes.bfloat16),
            "woT": woT,
            "bq": np.ascontiguousarray(
                np.asarray(bq, np.float32)[sl].reshape(NPAIR, P).T),
            "bk": np.ascontiguousarray(
                np.asarray(bk, np.float32)[sl].reshape(NPAIR, P).T),
            "bv": np.ascontiguousarray(
                np.asarray(bv, np.float32)[sl].reshape(NPAIR, P).T),
            "bo": bo_r,
        })
    return in_maps


def _assemble(results):
    yout = np.empty((B, S, D), np.float32)
    for d in range(NCORES):
        rows = slice(d * RB, (d + 1) * RB)
        yout[0, rows, :] = results[d]["y"][0:RB]
        yout[1, rows, :] = results[d]["y"][RB:2 * RB]
    return yout


def run_traced(trace=False, **inputs):
    """Run and return (output, BassKernelResults) — used by test.py."""
    nc = _get_nc()
    res = run_bass_kernel_spmd(
        nc, _make_in_maps(**inputs), core_ids=list(range(NCORES)), trace=trace
    )
    return _assemble(res.results), res


def kernel(**inputs) -> np.ndarray:
    out, _ = run_traced(trace=False, **inputs)
    return out


# revision 49
# speedup vs baseline: 1.3055x; 1.3055x over previous
"""Multi-head attention (B=2, S=2048, D=1024, H=16) on 8 trn2 NeuronCores.

Sharding: 2 groups of 4 cores; group b owns batch b, core (group rank r)
owns heads [4r:4r+4] (channels [256r:256r+256]). Each core loads only its
batch's x (8.4 MB), transposes it via the PE, projects q/k/v for its 4
heads, and runs attention. The output projection is re-sharded by sequence
rows: destination core d takes rows [256d:256d+256) of BOTH batches.

vs the 375us v1 baseline:
- x is PE-transposed in f32r (1.5 cyc/row); the PSUM->SBUF copy converts
  to bf16 so every downstream matmul runs uniform bf16 (weights/q/k/v/et/
  wo/h) at 1 col/cycle with FWL weight loads.  (Bulk DVE f32->bf16 casts
  measured ~5x below spec; mixed f32r x bf16 matmuls are rejected.)
- The AllToAll is split into 4 chunks overlapped with stage-B compute.
  qT is stored stripe-permuted (column 512g+64d+k holds token 256d+64g+k)
  so attention i-chunk g covers stripe g of EVERY destination; its A2A
  chunk fires while chunk g+1 computes.  Out-proj block 0 interleaves
  into the PE stream mid stage B; block 1's stripe-2 half computes inside
  the last A2A's ~20us latency window, leaving only its stripe-3 half +
  y writes serial (was ~67us of dead tail in v1).
- Per-head score tiles (one PSUM bank each, 4 bufs) + 4 separate attn@v
  accumulator banks: no pair-transition PE stalls (which also re-trigger
  the HAM 4/8 clock throttle).
- Softmax denominators come free from the [v|ones|v] attn@v trick; the
  per-pair normalize is 2 partition-shift copies + ONE exact DVE
  reciprocal (column-bound ~4us/512 cols) + 2 multiplies.
  reciprocal_approx_fast (custom-DVE table) computes garbage under this
  runtime; Ln/Exp on scalar thrashes ACT tables (~2.6us per switch).
- DMA: x split across the sync AND scalar HWDGE queues (~140GB/s each);
  gpsimd SWDGE carries biases -> weights -> bo in need-order (a 4KB bo
  load queued behind the weights once stalled the whole PE for 14us);
  warm-up transposes run during the first x-chunk wait (HAM clock gate
  needs ~3.4us of sustained PE activity to release full clock).

Matmuls keep a uniform K=128 stream: per-head d_k=64 scores contraction is
zero-padded to 128 (kT stored per-head with the sibling head's 64 rows
zeroed); mixing K=64/K=128 measurably degrades every matmul (~724ns vs
~227ns).
"""

import numpy as np

import concourse.bass as bass
import concourse.mybir as mybir
import concourse.tile as tile
from concourse import bacc
from concourse.masks import make_identity
from concourse.bass_utils import run_bass_kernel_spmd

# problem constants (hardcoded per harness contract)
B, S, D = 2, 2048, 1024
H, DK = 16, 64
NCORES = 8
GPB = NCORES // B          # cores per batch group = 4
HPC = H // NCORES * B      # heads per core = 4
NPAIR = HPC // 2           # head pairs per core = 2
CS = HPC * DK              # per-core channel slice = 256
TCH = 512                  # stage-A t-chunk
NTCH = S // TCH            # 4 chunks (one batch per core)
IB = 512                   # stage-B i-chunk
RB = 256                   # output rows per (core, batch)
NSTRIPE = 4                # A2A chunks == stage-B i-chunks
SW = RB // NSTRIPE         # stripe width = 64 rows per dest per chunk
P = 128
F32 = mybir.dt.float32
F32R = mybir.dt.float32r
BF16 = mybir.dt.bfloat16
AF = mybir.ActivationFunctionType
ALU = mybir.AluOpType


def build_nc():
    nc = bacc.Bacc("TRN2", target_bir_lowering=False, debug=False, num_devices=NCORES)

    x = nc.dram_tensor("x", [S, D], F32R, kind="ExternalInput")
    wqT = nc.dram_tensor("wqT", [D, CS], BF16, kind="ExternalInput")
    wkT = nc.dram_tensor("wkT", [D, CS], BF16, kind="ExternalInput")
    wvT = nc.dram_tensor("wvT", [D, CS], BF16, kind="ExternalInput")
    woT = nc.dram_tensor("woT", [D, D], BF16, kind="ExternalInput")
    bq = nc.dram_tensor("bq", [P, NPAIR], F32, kind="ExternalInput")
    bk = nc.dram_tensor("bk", [P, NPAIR], F32, kind="ExternalInput")
    bv = nc.dram_tensor("bv", [P, NPAIR], F32, kind="ExternalInput")
    bo = nc.dram_tensor("bo", [1, D], F32R, kind="ExternalInput")
    y = nc.dram_tensor("y", [2 * RB, D], F32, kind="ExternalOutput")

    with tile.TileContext(nc) as tc:
        with (
            tc.tile_pool(name="const", bufs=1) as cpool,
            tc.tile_pool(name="persist", bufs=1) as ppool,
            tc.tile_pool(name="dram", bufs=1, space="DRAM") as dpool,
        ):
            # identity first: the warm-up transposes and first x transposes
            # gate on it
            ident32 = cpool.tile([P, P], F32)
            make_identity(nc, ident32[:])
            identR_t = cpool.tile([P, P], F32R)
            nc.vector.tensor_copy(identR_t[:], ident32[:])
            identR = identR_t[:]
            identB = cpool.tile([P, P], BF16)
            nc.vector.tensor_copy(identB[:], ident32[:])

            # gpsimd SWDGE queue order = need order: small biases, then the
            # 1.5MB of weights (needed ~25us in), bo last (needed at stage C).
            # The queue is FIFO and moves only ~125GB/s.
            bq_sb = cpool.tile([P, NPAIR], F32)
            bk_sb = cpool.tile([P, NPAIR], F32)
            bv_sb = cpool.tile([P, NPAIR], F32)
            nc.gpsimd.dma_start(bq_sb[:], bq[:])
            nc.gpsimd.dma_start(bk_sb[:], bk[:])
            nc.gpsimd.dma_start(bv_sb[:], bv[:])
            wq_sb = cpool.tile([P, 8, CS], BF16)
            wk_sb = cpool.tile([P, 8, CS], BF16)
            wv_sb = cpool.tile([P, 8, CS], BF16)
            nc.gpsimd.dma_start(wq_sb[:], wqT[:].rearrange("(o p) c -> p o c", p=P))
            nc.gpsimd.dma_start(wk_sb[:], wkT[:].rearrange("(o p) c -> p o c", p=P))
            nc.gpsimd.dma_start(wv_sb[:], wvT[:].rearrange("(o p) c -> p o c", p=P))
            bo_sb = cpool.tile([1, D], F32R)
            nc.gpsimd.dma_start(bo_sb[:], bo[:])

            ones1 = cpool.tile([1, P], F32)
            nc.vector.memset(ones1[:], 1.0)
            ones_row = cpool.tile([1, P], F32R)
            nc.vector.tensor_copy(ones_row[:], ones1[:])
            bo_full = cpool.tile([P, D], F32)

            # persistent activations.  qT is stripe-permuted: [P, g, d, k]
            # where column (g, d, k) holds token 256d + 64g + k.
            qTq = [ppool.tile([P, NSTRIPE, NCORES, SW], BF16, name=f"qT{pp_}")
                   for pp_ in range(NPAIR)]
            kTh = [ppool.tile([P, S], BF16, name=f"kTh{h}") for h in range(HPC)]
            for h in range(HPC):
                z0 = (1 - h % 2) * DK  # zero rows: even head -> 64:128, odd -> 0:64
                nc.vector.memset(kTh[h][z0:z0 + DK, :], 0.0)
            # v per head-pair j-tile stored [v_even | ones | v_odd] (192 cols):
            # head h's attn@v lhsT is the contiguous 128-col slice at 64*(h%2);
            # the shared ones block replicates the softmax denominator onto the
            # other 64 output partitions.
            v_sbp = [ppool.tile([P, S // P, 3 * DK], BF16, name=f"v{pp_}")
                     for pp_ in range(NPAIR)]
            for pp_ in range(NPAIR):
                nc.vector.memset(v_sbp[pp_][:, :, DK:2 * DK], 1.0)

            # PE warm-up during the first x-chunk DMA wait: the HAM clock
            # gate needs ~3.4us of sustained activity to release 2.4 GHz.
            with tc.tile_pool(name="psW", bufs=2, space="PSUM") as psW:
                for w in range(6):
                    pw = psW.tile([P, 4 * P], BF16, tag="warm")
                    for t in range(4):
                        nc.tensor.transpose(
                            pw[:, t * P:(t + 1) * P], identB[:], identB[:])

            # ---- stage A: x cast+transpose + q/k/v projections ----
            with (
                tc.tile_pool(name="aw", bufs=1) as awpool,
                tc.tile_pool(name="vtc", bufs=2) as vtcpool,
                tc.tile_pool(name="stageA", bufs=2) as apool,
                tc.tile_pool(name="psA", bufs=2, space="PSUM") as psA,
                tc.tile_pool(name="psP", bufs=3, space="PSUM") as psP,
            ):
                for te in range(NTCH):
                    x_e = apool.tile([P, TCH // P, D], F32R, tag="x_e", bufs=3)
                    # split every chunk across the sync AND scalar HWDGE
                    # queues — one queue sustains only ~140GB/s, two nearly
                    # double the x feed rate
                    if te == 0:
                        # split the first load finer so transposes start sooner
                        for hh in range(4):
                            r0 = hh * (TCH // 4)
                            for qq in range(2):
                                eng = nc.sync if qq == 0 else nc.scalar
                                eng.dma_start(
                                    x_e[:, hh:hh + 1, qq * 512:(qq + 1) * 512],
                                    x[r0:r0 + TCH // 4,
                                      qq * 512:(qq + 1) * 512].rearrange(
                                        "(tt p) d -> p tt d", p=P
                                    ),
                                )
                    else:
                        for half in range(2):
                            eng = nc.sync if half == 0 else nc.scalar
                            eng.dma_start(
                                x_e[:, 2 * half:2 * half + 2, :],
                                x[te * TCH + half * (TCH // 2):
                                  te * TCH + (half + 1) * (TCH // 2), :
                                  ].rearrange("(tt p) d -> p tt d", p=P),
                            )
                    # f32r PE transposes (1.5 cyc/row); the PSUM->SBUF copy
                    # does the bf16 conversion so the projection matmuls get
                    # uniform bf16 operands (mixed f32r/bf16 is rejected, and
                    # a separate bulk DVE cast measured ~5x slower than spec)
                    xT_e = apool.tile([P, 8, TCH], BF16, tag="xT_e")
                    for dd in range(8):
                        ps = psA.tile([P, TCH], F32R, tag="trps")
                        for tt in range(TCH // P):
                            nc.tensor.transpose(
                                ps[:, tt * P:(tt + 1) * P],
                                x_e[:, tt, dd * P:(dd + 1) * P],
                                identR,
                            )
                        if dd % 2 == 0:
                            nc.vector.tensor_copy(xT_e[:, dd, :], ps[:])
                        else:
                            nc.scalar.activation(
                                xT_e[:, dd, :], ps[:], AF.Copy)
                    sl = slice(te * TCH, (te + 1) * TCH)
                    vTc = [None, None]
                    for proj, (w_sb, b_sb) in enumerate(
                        ((wq_sb, bq_sb), (wk_sb, bk_sb), (wv_sb, bv_sb))
                    ):
                        for ct in range(NPAIR):
                            cs = slice(ct * P, (ct + 1) * P)
                            pp = psP.tile([P, TCH], F32, tag="projps")
                            for dd in range(8):
                                nc.tensor.matmul(
                                    pp[:],
                                    w_sb[:, dd, cs],
                                    xT_e[:, dd, :],
                                    start=(dd == 0),
                                    stop=(dd == 7),
                                )
                            if proj == 0:
                                # stripe-permuted write: token 256dl+64g+k of
                                # this chunk -> qT[:, g, 2te+dl, k]
                                for dl in range(2):
                                    nc.vector.tensor_tensor(
                                        qTq[ct][:, :, 2 * te + dl, :],
                                        pp[:, dl * 256:(dl + 1) * 256].rearrange(
                                            "p (g k) -> p g k", g=NSTRIPE
                                        ),
                                        b_sb[:, ct:ct + 1, None].to_broadcast(
                                            [P, NSTRIPE, SW]),
                                        ALU.add,
                                    )
                            elif proj == 1:
                                for hh in range(2):
                                    hs = slice(hh * DK, (hh + 1) * DK)
                                    nc.vector.tensor_tensor(
                                        kTh[2 * ct + hh][hs, sl],
                                        pp[hs, :],
                                        b_sb[hs, ct:ct + 1
                                             ].to_broadcast([DK, TCH]),
                                        ALU.add,
                                    )
                            else:
                                vt = vtcpool.tile([P, TCH], BF16,
                                                  tag=f"vTc{ct}", name=f"vTc{ct}")
                                nc.vector.tensor_tensor(
                                    vt[:], pp[:],
                                    b_sb[:, ct:ct + 1].to_broadcast([P, TCH]),
                                    ALU.add,
                                )
                                vTc[ct] = vt
                    # vT [c, t] -> v natural [t, c] split around the ones block
                    for pp_ in range(NPAIR):
                        for tt in range(TCH // P):
                            psv = psA.tile([P, P], BF16, tag="vtr")
                            nc.tensor.transpose(
                                psv[:], vTc[pp_][:, tt * P:(tt + 1) * P],
                                identB[:])
                            ti = te * (TCH // P) + tt
                            # plain Copy on scalar is fast (unlike its
                            # bias-add path); keeps DVE for the bias-adds
                            nc.scalar.activation(
                                v_sbp[pp_][:, ti, 0:DK], psv[:, 0:DK],
                                AF.Copy)
                            nc.scalar.activation(
                                v_sbp[pp_][:, ti, 2 * DK:3 * DK],
                                psv[:, DK:2 * DK], AF.Copy)

            # ---- stage B (attention) with chunked A2A + interleaved
            # stage C (output projection) ----
            with (
                tc.tile_pool(name="stageC", bufs=1) as c2pool,
                tc.tile_pool(name="yout", bufs=2) as ypool,
            ):
                # the 2 MB wo load overlaps stage-B compute on the sync queue
                wo_sb = c2pool.tile([P, 8, D], BF16)
                nc.sync.dma_start(wo_sb[:], woT[:].rearrange("(o p) n -> p o n", p=P))
                # h layout [p, batch, src, o2, i]: channel 256*src+128*o2+p
                h_sb = c2pool.tile([P, 2, GPB, 2, 2 * P], BF16)

                # bo broadcast to all partitions via two K=1 matmuls, emitted
                # here (bo isn't needed until stage C; emitting it at the top
                # made the whole PE queue wait ~27us for the bo DMA)
                with tc.tile_pool(name="psbo", bufs=1, space="PSUM") as psbo:
                    for nch in range(D // 512):
                        pb = psbo.tile([P, 512], F32, tag="pb")
                        nc.tensor.matmul(
                            pb[:], ones_row[:],
                            bo_sb[:, nch * 512:(nch + 1) * 512],
                            start=True, stop=True,
                        )
                        nc.vector.tensor_copy(
                            bo_full[:, nch * 512:(nch + 1) * 512], pb[:])

                a2a_in = [dpool.tile([NCORES, CS, SW], BF16, name=f"a2ai{g}")
                          for g in range(NSTRIPE - 1)]
                a2a_out = [dpool.tile([NCORES, CS, SW], BF16, name=f"a2ao{g}")
                           for g in range(NSTRIPE - 1)]
                # the last chunk's A2A is split by head-pair: pair 0's half
                # fires ~18us before pair 1 finishes, so half the final
                # out-proj K-tiles can accumulate before the last collective
                a2a_in3 = [dpool.tile([NCORES, P, SW], BF16, name=f"a2ai3{q}")
                           for q in range(NPAIR)]
                a2a_out3 = [dpool.tile([NCORES, P, SW], BF16, name=f"a2ao3{q}")
                            for q in range(NPAIR)]

                with (
                    tc.tile_pool(name="et", bufs=6) as etpool,
                    tc.tile_pool(name="ob", bufs=2) as obpool,
                    tc.tile_pool(name="psS", bufs=4, space="PSUM") as psS,
                    tc.tile_pool(name="psAV", bufs=1, space="PSUM") as psAV,
                ):
                    def load_h_stripe(g):
                        # pull A2A chunk g's payload into h_sb columns
                        # [64g, 64g+64) — emitted once chunk g's collective
                        # is safely complete so the sync queue never blocks
                        it, st = g // 2, g % 2
                        c0 = it * P + st * SW
                        for hb in range(2):
                            nc.sync.dma_start(
                                h_sb[:, hb, :, :, c0:c0 + SW],
                                a2a_out[g][
                                    hb * GPB:(hb + 1) * GPB, :, :
                                ].rearrange("s (o2 p) k -> p s o2 k", p=P),
                            )

                    def load_h3(q):
                        # stripe 3, pair q's channels (o2=q), from its own
                        # half-collective; split across both HWDGE queues
                        c0 = P + SW
                        for hb in range(2):
                            eng = nc.sync if hb == 0 else nc.scalar
                            eng.dma_start(
                                h_sb[:, hb, :, q, c0:c0 + SW],
                                a2a_out3[q][
                                    hb * GPB:(hb + 1) * GPB, :, :
                                ].rearrange("s p k -> p s k"),
                            )

                    def stage_c_block(it):
                        # out-proj for my rows [128it:128it+128) of each batch
                        for hb in range(2):
                            y_sb = ypool.tile([P, D], F32, tag="y")
                            for nch in range(2):
                                ns = slice(nch * 512, (nch + 1) * 512)
                                py = psS.tile([P, IB], F32, tag="s", name="py")
                                for o in range(8):
                                    nc.tensor.matmul(
                                        py[:],
                                        h_sb[:, hb, o // 2, o % 2,
                                             it * P:(it + 1) * P],
                                        wo_sb[:, o, ns],
                                        start=(o == 0),
                                        stop=(o == 7),
                                    )
                                nc.vector.tensor_tensor(
                                    y_sb[:, ns], py[:], bo_full[:, ns], ALU.add)
                            r0 = hb * RB + it * P
                            nc.sync.dma_start(y[r0:r0 + P, :], y_sb[:])

                    for g in range(NSTRIPE):
                        for pp_ in range(NPAIR):
                            # 4 distinct accumulator banks: no pair-transition
                            # stall waiting for the previous pair's normalize
                            av_e = psAV.tile([P, IB], F32, tag=f"av{2 * pp_}",
                                             name=f"av{2 * pp_}")
                            av_o = psAV.tile([P, IB], F32, tag=f"av{2 * pp_ + 1}",
                                             name=f"av{2 * pp_ + 1}")
                            qTg = qTq[pp_][:, g].rearrange("p d k -> p (d k)")
                            for jt in range(S // P):
                                j0 = jt * P
                                et = etpool.tile([P, 2 * IB], BF16, tag="et")
                                for hh in range(2):
                                    sps = psS.tile([P, IB], F32, tag="s",
                                                   name=f"sps{hh}")
                                    nc.tensor.matmul(
                                        sps[:],
                                        kTh[2 * pp_ + hh][:, j0:j0 + P],
                                        qTg,
                                        start=True,
                                        stop=True,
                                    )
                                    nc.scalar.activation(
                                        et[:, hh * IB:(hh + 1) * IB],
                                        sps[:], AF.Exp, scale=0.125)
                                for hh, av in ((0, av_e), (1, av_o)):
                                    nc.tensor.matmul(
                                        av[:],
                                        v_sbp[pp_][:, jt, hh * DK:hh * DK + 2 * DK],
                                        et[:, hh * IB:(hh + 1) * IB],
                                        start=(jt == 0),
                                        stop=(jt == S // P - 1),
                                    )
                            # pair-merged normalize: even head raw rows 0:64 /
                            # denom 64:128; odd head flipped.  The 4 av banks
                            # keep this whole chain off the PE critical path.
                            # (Ln/Exp-on-scalar thrashes ACT tables ~2.6us per
                            # use; reciprocal_approx_fast returns wrong
                            # results under this runtime; DVE reciprocal is
                            # column-bound so merge both heads into ONE call.)
                            den = obpool.tile([P, IB], F32, tag="den")
                            nc.vector.tensor_copy(den[0:DK, :], av_e[DK:P, :])
                            nc.vector.tensor_copy(den[DK:P, :], av_o[0:DK, :])
                            rec = obpool.tile([P, IB], F32, tag="rec")
                            nc.vector.reciprocal(rec[:], den[:])
                            onrm = obpool.tile([P, IB], BF16, tag="onrm")
                            nc.vector.tensor_tensor(
                                onrm[0:DK, :], av_e[0:DK, :], rec[0:DK, :],
                                ALU.mult)
                            nc.vector.tensor_tensor(
                                onrm[DK:P, :], av_o[DK:P, :], rec[DK:P, :],
                                ALU.mult)
                            # chunk-local column (d, k) -> dest core d, row k
                            if g < NSTRIPE - 1:
                                nc.sync.dma_start(
                                    a2a_in[g][:, pp_ * P:(pp_ + 1) * P, :
                                              ].rearrange("d p k -> p d k"),
                                    onrm[:].rearrange("p (d k) -> p d k",
                                                      d=NCORES),
                                )
                            else:
                                # last chunk: per-pair half-collectives
                                nc.sync.dma_start(
                                    a2a_in3[pp_][:].rearrange("d p k -> p d k"),
                                    onrm[:].rearrange("p (d k) -> p d k",
                                                      d=NCORES),
                                )
                                nc.gpsimd.collective_compute(
                                    "AllToAll",
                                    ALU.bypass,
                                    replica_groups=[list(range(NCORES))],
                                    ins=[a2a_in3[pp_].opt()],
                                    outs=[a2a_out3[pp_].opt()],
                                )
                        if g < NSTRIPE - 1:
                            nc.gpsimd.collective_compute(
                                "AllToAll",
                                ALU.bypass,
                                replica_groups=[list(range(NCORES))],
                                ins=[a2a_in[g].opt()],
                                outs=[a2a_out[g].opt()],
                            )
                        # chunk g-1's collective finished while chunk g
                        # computed: its h-load won't block the sync queue
                        if g >= 1:
                            load_h_stripe(g - 1)
                        if g == 2:
                            # chunks 0+1 landed while chunk 2 computed
                            stage_c_block(0)
                    # ---- tail: block 1's out-proj split by K-tile parity.
                    # Pair-0's half-collective landed ~18us early, so the
                    # o2=0 K-tiles accumulate BEFORE the last collective;
                    # only the o2=1 pass + y writes remain after it. ----
                    load_h3(0)
                    it = 1
                    pys = {}
                    for hb in range(2):
                        for nch in range(2):
                            ns = slice(nch * 512, (nch + 1) * 512)
                            py = psS.tile([P, IB], F32, tag="s",
                                          name=f"py{hb}{nch}")
                            pys[hb, nch] = py
                            for s in range(GPB):
                                nc.tensor.matmul(
                                    py[:],
                                    h_sb[:, hb, s, 0, it * P:(it + 1) * P],
                                    wo_sb[:, 2 * s, ns],
                                    start=(s == 0),
                                    stop=False,
                                )
                    # keep the PE's HAM clock gate from re-throttling during
                    # the last collective's ~15us latency window
                    for w in range(20):
                        pw = psAV.tile([P, IB], F32, tag=f"av{w % 4}",
                                       name="pw")
                        nc.tensor.matmul(pw[:], identB[:], wo_sb[:, 0, 0:IB],
                                         start=True, stop=True)
                    load_h3(1)
                    for hb in range(2):
                        y_sb = ypool.tile([P, D], F32, tag="y")
                        for nch in range(2):
                            ns = slice(nch * 512, (nch + 1) * 512)
                            py = pys[hb, nch]
                            for s in range(GPB):
                                nc.tensor.matmul(
                                    py[:],
                                    h_sb[:, hb, s, 1, it * P:(it + 1) * P],
                                    wo_sb[:, 2 * s + 1, ns],
                                    start=False,
                                    stop=(s == GPB - 1),
                                )
                            nc.vector.tensor_tensor(
                                y_sb[:, ns], py[:], bo_full[:, ns], ALU.add)
                        r0 = hb * RB + it * P
                        for half in range(2):
                            eng = nc.sync if half == 0 else nc.scalar
                            eng.dma_start(
                                y[r0:r0 + P, half * 512:(half + 1) * 512],
                                y_sb[:, half * 512:(half + 1) * 512],
                            )

    nc.compile()
    return nc


_NC = None


def _get_nc():
    global _NC
    if _NC is None:
        _NC = build_nc()
    return _NC


def _make_in_maps(x, Wq, bq, Wk, bk, Wv, bv, Wo, bo):
    import ml_dtypes
    xf = np.asarray(x, np.float32).reshape(B, S, D)
    woT = np.ascontiguousarray(np.asarray(Wo, np.float32).T).astype(ml_dtypes.bfloat16)
    bo_r = np.ascontiguousarray(np.asarray(bo, np.float32).reshape(1, D))
    Wq = np.asarray(Wq, np.float32)
    Wk = np.asarray(Wk, np.float32)
    Wv = np.asarray(Wv, np.float32)
    in_maps = []
    for c in range(NCORES):
        b = c // GPB
        r = c % GPB
        sl = slice(r * CS, (r + 1) * CS)
        in_maps.append({
            "x": np.ascontiguousarray(xf[b]),
            "wqT": np.ascontiguousarray(Wq[sl, :].T).astype(ml_dtypes.bfloat16),
            "wkT": np.ascontiguousarray(Wk[sl, :].T).astype(ml_dtypes.bfloat16),
            "wvT": np.ascontiguousarray(Wv[sl, :].T).astype(ml_dtypes.bfloat16),
            "woT": woT,
            "bq": np.ascontiguousarray(
                np.asarray(bq, np.float32)[sl].reshape(NPAIR, P).T),
            "bk": np.ascontiguousarray(
                np.asarray(bk, np.float32)[sl].reshape(NPAIR, P).T),
            "bv": np.ascontiguousarray(
                np.asarray(bv, np.float32)[sl].reshape(NPAIR, P).T),
            "bo": bo_r,
        })
    return in_maps


def _assemble(results):
    yout = np.empty((B, S, D), np.float32)
    for d in range(NCORES):
        rows = slice(d * RB, (d + 1) * RB)
        yout[0, rows, :] = results[d]["y"][0:RB]
        yout[1, rows, :] = results[d]["y"][RB:2 * RB]
    return yout


def run_traced(trace=False, **inputs):
    """Run and return (output, BassKernelResults) — used by test.py."""
    nc = _get_nc()
    res = run_bass_kernel_spmd(
        nc, _make_in_maps(**inputs), core_ids=list(range(NCORES)), trace=trace
    )
    return _assemble(res.results), res


def kernel(**inputs) -> np.ndarray:
    out, _ = run_traced(trace=False, **inputs)
    return out


# revision 53
# speedup vs baseline: 1.3323x; 1.0205x over previous
"""Multi-head attention (B=2, S=2048, D=1024, H=16) on 8 trn2 NeuronCores.

Sharding: 2 groups of 4 cores; group b owns batch b, core (group rank r)
owns heads [4r:4r+4] (channels [256r:256r+256]). Each core loads only its
batch's x (8.4 MB), transposes it via the PE, projects q/k/v for its 4
heads, and runs attention. The output projection is re-sharded by sequence
rows: destination core d takes rows [256d:256d+256) of BOTH batches.

vs the 375us v1 baseline:
- x is PE-transposed in f32r (1.5 cyc/row); the PSUM->SBUF copy converts
  to bf16 so every downstream matmul runs uniform bf16 (weights/q/k/v/et/
  wo/h) at 1 col/cycle with FWL weight loads.  (Bulk DVE f32->bf16 casts
  measured ~5x below spec; mixed f32r x bf16 matmuls are rejected.)
- The AllToAll is split into 4 chunks overlapped with stage-B compute.
  qT is stored stripe-permuted (column 512g+64d+k holds token 256d+64g+k)
  so attention i-chunk g covers stripe g of EVERY destination; its A2A
  chunk fires while chunk g+1 computes.  Out-proj block 0 interleaves
  into the PE stream mid stage B; block 1's stripe-2 half computes inside
  the last A2A's ~20us latency window, leaving only its stripe-3 half +
  y writes serial (was ~67us of dead tail in v1).
- Per-head score tiles (one PSUM bank each, 4 bufs) + 4 separate attn@v
  accumulator banks: no pair-transition PE stalls (which also re-trigger
  the HAM 4/8 clock throttle).
- Softmax denominators come free from the [v|ones|v] attn@v trick; the
  per-pair normalize is 2 partition-shift copies + ONE exact DVE
  reciprocal (column-bound ~4us/512 cols) + 2 multiplies.
  reciprocal_approx_fast (custom-DVE table) computes garbage under this
  runtime; Ln/Exp on scalar thrashes ACT tables (~2.6us per switch).
- DMA: x split across the sync AND scalar HWDGE queues (~140GB/s each);
  gpsimd SWDGE carries biases -> weights -> bo in need-order (a 4KB bo
  load queued behind the weights once stalled the whole PE for 14us);
  warm-up transposes run during the first x-chunk wait (HAM clock gate
  needs ~3.4us of sustained PE activity to release full clock).

Matmuls keep a uniform K=128 stream: per-head d_k=64 scores contraction is
zero-padded to 128 (kT stored per-head with the sibling head's 64 rows
zeroed); mixing K=64/K=128 measurably degrades every matmul (~724ns vs
~227ns).
"""

import numpy as np

import concourse.bass as bass
import concourse.mybir as mybir
import concourse.tile as tile
from concourse import bacc
from concourse.masks import make_identity
from concourse.bass_utils import run_bass_kernel_spmd

# problem constants (hardcoded per harness contract)
B, S, D = 2, 2048, 1024
H, DK = 16, 64
NCORES = 8
GPB = NCORES // B          # cores per batch group = 4
HPC = H // NCORES * B      # heads per core = 4
NPAIR = HPC // 2           # head pairs per core = 2
CS = HPC * DK              # per-core channel slice = 256
TCH = 512                  # stage-A t-chunk
NTCH = S // TCH            # 4 chunks (one batch per core)
IB = 512                   # stage-B i-chunk
RB = 256                   # output rows per (core, batch)
NSTRIPE = 4                # A2A chunks == stage-B i-chunks
SW = RB // NSTRIPE         # stripe width = 64 rows per dest per chunk
P = 128
F32 = mybir.dt.float32
F32R = mybir.dt.float32r
BF16 = mybir.dt.bfloat16
AF = mybir.ActivationFunctionType
ALU = mybir.AluOpType


def build_nc():
    nc = bacc.Bacc("TRN2", target_bir_lowering=False, debug=False, num_devices=NCORES)

    x = nc.dram_tensor("x", [S, D], F32R, kind="ExternalInput")
    wqT = nc.dram_tensor("wqT", [D, CS], BF16, kind="ExternalInput")
    wkT = nc.dram_tensor("wkT", [D, CS], BF16, kind="ExternalInput")
    wvT = nc.dram_tensor("wvT", [D, CS], BF16, kind="ExternalInput")
    woT = nc.dram_tensor("woT", [D, D], BF16, kind="ExternalInput")
    bq = nc.dram_tensor("bq", [P, NPAIR], F32, kind="ExternalInput")
    bk = nc.dram_tensor("bk", [P, NPAIR], F32, kind="ExternalInput")
    bv = nc.dram_tensor("bv", [P, NPAIR], F32, kind="ExternalInput")
    bo = nc.dram_tensor("bo", [1, D], F32R, kind="ExternalInput")
    y = nc.dram_tensor("y", [2 * RB, D], F32, kind="ExternalOutput")

    with tile.TileContext(nc) as tc:
        with (
            tc.tile_pool(name="const", bufs=1) as cpool,
            tc.tile_pool(name="persist", bufs=1) as ppool,
            tc.tile_pool(name="dram", bufs=1, space="DRAM") as dpool,
        ):
            # identity first: the warm-up transposes and first x transposes
            # gate on it
            ident32 = cpool.tile([P, P], F32)
            make_identity(nc, ident32[:])
            identR_t = cpool.tile([P, P], F32R)
            nc.vector.tensor_copy(identR_t[:], ident32[:])
            identR = identR_t[:]
            identB = cpool.tile([P, P], BF16)
            nc.vector.tensor_copy(identB[:], ident32[:])

            # gpsimd SWDGE queue order = need order: small biases, then the
            # 1.5MB of weights (needed ~25us in), bo last (needed at stage C).
            # The queue is FIFO and moves only ~125GB/s.
            bq_sb = cpool.tile([P, NPAIR], F32)
            bk_sb = cpool.tile([P, NPAIR], F32)
            bv_sb = cpool.tile([P, NPAIR], F32)
            nc.gpsimd.dma_start(bq_sb[:], bq[:])
            nc.gpsimd.dma_start(bk_sb[:], bk[:])
            nc.gpsimd.dma_start(bv_sb[:], bv[:])
            wq_sb = cpool.tile([P, 8, CS], BF16)
            wk_sb = cpool.tile([P, 8, CS], BF16)
            wv_sb = cpool.tile([P, 8, CS], BF16)
            nc.gpsimd.dma_start(wq_sb[:], wqT[:].rearrange("(o p) c -> p o c", p=P))
            nc.gpsimd.dma_start(wk_sb[:], wkT[:].rearrange("(o p) c -> p o c", p=P))
            nc.gpsimd.dma_start(wv_sb[:], wvT[:].rearrange("(o p) c -> p o c", p=P))
            bo_sb = cpool.tile([1, D], F32R)
            nc.gpsimd.dma_start(bo_sb[:], bo[:])

            ones1 = cpool.tile([1, P], F32)
            nc.vector.memset(ones1[:], 1.0)
            ones_row = cpool.tile([1, P], F32R)
            nc.vector.tensor_copy(ones_row[:], ones1[:])
            bo_full = cpool.tile([P, D], F32)

            # persistent activations.  qT is stripe-permuted: [P, g, d, k]
            # where column (g, d, k) holds token 256d + 64g + k.
            qTq = [ppool.tile([P, NSTRIPE, NCORES, SW], BF16, name=f"qT{pp_}")
                   for pp_ in range(NPAIR)]
            kTh = [ppool.tile([P, S], BF16, name=f"kTh{h}") for h in range(HPC)]
            for h in range(HPC):
                z0 = (1 - h % 2) * DK  # zero rows: even head -> 64:128, odd -> 0:64
                nc.vector.memset(kTh[h][z0:z0 + DK, :], 0.0)
            # v per head-pair j-tile stored [v_even | ones | v_odd] (192 cols):
            # head h's attn@v lhsT is the contiguous 128-col slice at 64*(h%2);
            # the shared ones block replicates the softmax denominator onto the
            # other 64 output partitions.
            v_sbp = [ppool.tile([P, S // P, 3 * DK], BF16, name=f"v{pp_}")
                     for pp_ in range(NPAIR)]
            for pp_ in range(NPAIR):
                nc.vector.memset(v_sbp[pp_][:, :, DK:2 * DK], 1.0)

            # PE warm-up during the first x-chunk DMA wait: the HAM clock
            # gate needs ~3.4us of sustained activity to release 2.4 GHz.
            with tc.tile_pool(name="psW", bufs=2, space="PSUM") as psW:
                for w in range(10):
                    pw = psW.tile([P, 4 * P], BF16, tag="warm")
                    for t in range(4):
                        nc.tensor.transpose(
                            pw[:, t * P:(t + 1) * P], identB[:], identB[:])

            # ---- stage A: x cast+transpose + q/k/v projections ----
            with (
                tc.tile_pool(name="aw", bufs=1) as awpool,
                tc.tile_pool(name="vtc", bufs=2) as vtcpool,
                tc.tile_pool(name="stageA", bufs=2) as apool,
                tc.tile_pool(name="psA", bufs=2, space="PSUM") as psA,
                tc.tile_pool(name="psP", bufs=3, space="PSUM") as psP,
            ):
                for te in range(NTCH):
                    x_e = apool.tile([P, TCH // P, D], F32R, tag="x_e", bufs=3)
                    # split every chunk across the sync AND scalar HWDGE
                    # queues — one queue sustains only ~140GB/s, two nearly
                    # double the x feed rate
                    if te == 0:
                        # split the first load finer so transposes start sooner
                        for hh in range(4):
                            r0 = hh * (TCH // 4)
                            for qq in range(2):
                                eng = nc.sync if qq == 0 else nc.scalar
                                eng.dma_start(
                                    x_e[:, hh:hh + 1, qq * 512:(qq + 1) * 512],
                                    x[r0:r0 + TCH // 4,
                                      qq * 512:(qq + 1) * 512].rearrange(
                                        "(tt p) d -> p tt d", p=P
                                    ),
                                )
                    else:
                        for half in range(2):
                            eng = nc.sync if half == 0 else nc.scalar
                            eng.dma_start(
                                x_e[:, 2 * half:2 * half + 2, :],
                                x[te * TCH + half * (TCH // 2):
                                  te * TCH + (half + 1) * (TCH // 2), :
                                  ].rearrange("(tt p) d -> p tt d", p=P),
                            )
                    # f32r PE transposes (1.5 cyc/row); the PSUM->SBUF copy
                    # does the bf16 conversion so the projection matmuls get
                    # uniform bf16 operands (mixed f32r/bf16 is rejected, and
                    # a separate bulk DVE cast measured ~5x slower than spec)
                    xT_e = apool.tile([P, 8, TCH], BF16, tag="xT_e")
                    for dd in range(8):
                        ps = psA.tile([P, TCH], F32R, tag="trps")
                        for tt in range(TCH // P):
                            nc.tensor.transpose(
                                ps[:, tt * P:(tt + 1) * P],
                                x_e[:, tt, dd * P:(dd + 1) * P],
                                identR,
                            )
                        if dd % 2 == 0:
                            nc.vector.tensor_copy(xT_e[:, dd, :], ps[:])
                        else:
                            nc.scalar.activation(
                                xT_e[:, dd, :], ps[:], AF.Copy)
                    sl = slice(te * TCH, (te + 1) * TCH)
                    vTc = [None, None]
                    for proj, (w_sb, b_sb) in enumerate(
                        ((wq_sb, bq_sb), (wk_sb, bk_sb), (wv_sb, bv_sb))
                    ):
                        for ct in range(NPAIR):
                            cs = slice(ct * P, (ct + 1) * P)
                            pp = psP.tile([P, TCH], F32, tag="projps")
                            for dd in range(8):
                                nc.tensor.matmul(
                                    pp[:],
                                    w_sb[:, dd, cs],
                                    xT_e[:, dd, :],
                                    start=(dd == 0),
                                    stop=(dd == 7),
                                )
                            if proj == 0:
                                # stripe-permuted write: token 256dl+64g+k of
                                # this chunk -> qT[:, g, 2te+dl, k]
                                for dl in range(2):
                                    nc.vector.tensor_tensor(
                                        qTq[ct][:, :, 2 * te + dl, :],
                                        pp[:, dl * 256:(dl + 1) * 256].rearrange(
                                            "p (g k) -> p g k", g=NSTRIPE
                                        ),
                                        b_sb[:, ct:ct + 1, None].to_broadcast(
                                            [P, NSTRIPE, SW]),
                                        ALU.add,
                                    )
                            elif proj == 1:
                                for hh in range(2):
                                    hs = slice(hh * DK, (hh + 1) * DK)
                                    nc.vector.tensor_tensor(
                                        kTh[2 * ct + hh][hs, sl],
                                        pp[hs, :],
                                        b_sb[hs, ct:ct + 1
                                             ].to_broadcast([DK, TCH]),
                                        ALU.add,
                                    )
                            else:
                                vt = vtcpool.tile([P, TCH], BF16,
                                                  tag=f"vTc{ct}", name=f"vTc{ct}")
                                nc.vector.tensor_tensor(
                                    vt[:], pp[:],
                                    b_sb[:, ct:ct + 1].to_broadcast([P, TCH]),
                                    ALU.add,
                                )
                                vTc[ct] = vt
                    # vT [c, t] -> v natural [t, c] split around the ones block
                    for pp_ in range(NPAIR):
                        for tt in range(TCH // P):
                            psv = psA.tile([P, P], BF16, tag="vtr")
                            nc.tensor.transpose(
                                psv[:], vTc[pp_][:, tt * P:(tt + 1) * P],
                                identB[:])
                            ti = te * (TCH // P) + tt
                            # plain Copy on scalar is fast (unlike its
                            # bias-add path); keeps DVE for the bias-adds
                            nc.scalar.activation(
                                v_sbp[pp_][:, ti, 0:DK], psv[:, 0:DK],
                                AF.Copy)
                            nc.scalar.activation(
                                v_sbp[pp_][:, ti, 2 * DK:3 * DK],
                                psv[:, DK:2 * DK], AF.Copy)

            # ---- stage B (attention) with chunked A2A + interleaved
            # stage C (output projection) ----
            with (
                tc.tile_pool(name="stageC", bufs=1) as c2pool,
                tc.tile_pool(name="yout", bufs=2) as ypool,
            ):
                # the 2 MB wo load overlaps stage-B compute on the sync queue
                wo_sb = c2pool.tile([P, 8, D], BF16)
                nc.sync.dma_start(wo_sb[:], woT[:].rearrange("(o p) n -> p o n", p=P))
                # h layout [p, batch, src, o2, i]: channel 256*src+128*o2+p
                h_sb = c2pool.tile([P, 2, GPB, 2, 2 * P], BF16)

                # bo broadcast to all partitions via two K=1 matmuls, emitted
                # here (bo isn't needed until stage C; emitting it at the top
                # made the whole PE queue wait ~27us for the bo DMA)
                with tc.tile_pool(name="psbo", bufs=1, space="PSUM") as psbo:
                    for nch in range(D // 512):
                        pb = psbo.tile([P, 512], F32, tag="pb")
                        nc.tensor.matmul(
                            pb[:], ones_row[:],
                            bo_sb[:, nch * 512:(nch + 1) * 512],
                            start=True, stop=True,
                        )
                        nc.vector.tensor_copy(
                            bo_full[:, nch * 512:(nch + 1) * 512], pb[:])

                a2a_in = [dpool.tile([NCORES, CS, SW], BF16, name=f"a2ai{g}")
                          for g in range(NSTRIPE - 1)]
                a2a_out = [dpool.tile([NCORES, CS, SW], BF16, name=f"a2ao{g}")
                           for g in range(NSTRIPE - 1)]
                # the last chunk's A2A is split by head-pair: pair 0's half
                # fires ~18us before pair 1 finishes, so half the final
                # out-proj K-tiles can accumulate before the last collective
                a2a_in3 = [dpool.tile([NCORES, P, SW], BF16, name=f"a2ai3{q}")
                           for q in range(NPAIR)]
                a2a_out3 = [dpool.tile([NCORES, P, SW], BF16, name=f"a2ao3{q}")
                            for q in range(NPAIR)]

                with (
                    tc.tile_pool(name="et", bufs=6) as etpool,
                    tc.tile_pool(name="ob", bufs=2) as obpool,
                    tc.tile_pool(name="psS", bufs=4, space="PSUM") as psS,
                    tc.tile_pool(name="psAV", bufs=1, space="PSUM") as psAV,
                ):
                    def load_h_stripe(g):
                        # pull A2A chunk g's payload into h_sb columns
                        # [64g, 64g+64) — emitted once chunk g's collective
                        # is safely complete so the sync queue never blocks
                        it, st = g // 2, g % 2
                        c0 = it * P + st * SW
                        for hb in range(2):
                            nc.sync.dma_start(
                                h_sb[:, hb, :, :, c0:c0 + SW],
                                a2a_out[g][
                                    hb * GPB:(hb + 1) * GPB, :, :
                                ].rearrange("s (o2 p) k -> p s o2 k", p=P),
                            )

                    def load_h3(q):
                        # stripe 3, pair q's channels (o2=q), from its own
                        # half-collective; split across both HWDGE queues
                        c0 = P + SW
                        for hb in range(2):
                            eng = nc.sync if hb == 0 else nc.scalar
                            eng.dma_start(
                                h_sb[:, hb, :, q, c0:c0 + SW],
                                a2a_out3[q][
                                    hb * GPB:(hb + 1) * GPB, :, :
                                ].rearrange("s p k -> p s k"),
                            )

                    def stage_c_block(it):
                        # out-proj for my rows [128it:128it+128) of each batch
                        for hb in range(2):
                            y_sb = ypool.tile([P, D], F32, tag="y")
                            for nch in range(2):
                                ns = slice(nch * 512, (nch + 1) * 512)
                                py = psS.tile([P, IB], F32, tag="s", name="py")
                                for o in range(8):
                                    nc.tensor.matmul(
                                        py[:],
                                        h_sb[:, hb, o // 2, o % 2,
                                             it * P:(it + 1) * P],
                                        wo_sb[:, o, ns],
                                        start=(o == 0),
                                        stop=(o == 7),
                                    )
                                nc.vector.tensor_tensor(
                                    y_sb[:, ns], py[:], bo_full[:, ns], ALU.add)
                            r0 = hb * RB + it * P
                            nc.sync.dma_start(y[r0:r0 + P, :], y_sb[:])

                    for g in range(NSTRIPE):
                        for pp_ in range(NPAIR):
                            # 4 distinct accumulator banks: no pair-transition
                            # stall waiting for the previous pair's normalize
                            av_e = psAV.tile([P, IB], F32, tag=f"av{2 * pp_}",
                                             name=f"av{2 * pp_}")
                            av_o = psAV.tile([P, IB], F32, tag=f"av{2 * pp_ + 1}",
                                             name=f"av{2 * pp_ + 1}")
                            qTg = qTq[pp_][:, g].rearrange("p d k -> p (d k)")
                            for jt in range(S // P):
                                j0 = jt * P
                                et = etpool.tile([P, 2 * IB], BF16, tag="et")
                                for hh in range(2):
                                    sps = psS.tile([P, IB], F32, tag="s",
                                                   name=f"sps{hh}")
                                    nc.tensor.matmul(
                                        sps[:],
                                        kTh[2 * pp_ + hh][:, j0:j0 + P],
                                        qTg,
                                        start=True,
                                        stop=True,
                                    )
                                    nc.scalar.activation(
                                        et[:, hh * IB:(hh + 1) * IB],
                                        sps[:], AF.Exp, scale=0.125)
                                for hh, av in ((0, av_e), (1, av_o)):
                                    nc.tensor.matmul(
                                        av[:],
                                        v_sbp[pp_][:, jt, hh * DK:hh * DK + 2 * DK],
                                        et[:, hh * IB:(hh + 1) * IB],
                                        start=(jt == 0),
                                        stop=(jt == S // P - 1),
                                    )
                            # pair-merged normalize: even head raw rows 0:64 /
                            # denom 64:128; odd head flipped.  The 4 av banks
                            # keep this whole chain off the PE critical path.
                            # (Ln/Exp-on-scalar thrashes ACT tables ~2.6us per
                            # use; reciprocal_approx_fast returns wrong
                            # results under this runtime; DVE reciprocal is
                            # column-bound so merge both heads into ONE call.)
                            den = obpool.tile([P, IB], F32, tag="den")
                            nc.vector.tensor_copy(den[0:DK, :], av_e[DK:P, :])
                            nc.vector.tensor_copy(den[DK:P, :], av_o[0:DK, :])
                            rec = obpool.tile([P, IB], F32, tag="rec")
                            nc.vector.reciprocal(rec[:], den[:])
                            onrm = obpool.tile([P, IB], BF16, tag="onrm")
                            nc.vector.tensor_tensor(
                                onrm[0:DK, :], av_e[0:DK, :], rec[0:DK, :],
                                ALU.mult)
                            nc.vector.tensor_tensor(
                                onrm[DK:P, :], av_o[DK:P, :], rec[DK:P, :],
                                ALU.mult)
                            # chunk-local column (d, k) -> dest core d, row k
                            if g < NSTRIPE - 1:
                                nc.sync.dma_start(
                                    a2a_in[g][:, pp_ * P:(pp_ + 1) * P, :
                                              ].rearrange("d p k -> p d k"),
                                    onrm[:].rearrange("p (d k) -> p d k",
                                                      d=NCORES),
                                )
                            else:
                                # last chunk: per-pair half-collectives
                                nc.sync.dma_start(
                                    a2a_in3[pp_][:].rearrange("d p k -> p d k"),
                                    onrm[:].rearrange("p (d k) -> p d k",
                                                      d=NCORES),
                                )
                                nc.gpsimd.collective_compute(
                                    "AllToAll",
                                    ALU.bypass,
                                    replica_groups=[list(range(NCORES))],
                                    ins=[a2a_in3[pp_].opt()],
                                    outs=[a2a_out3[pp_].opt()],
                                )
                        if g < NSTRIPE - 1:
                            nc.gpsimd.collective_compute(
                                "AllToAll",
                                ALU.bypass,
                                replica_groups=[list(range(NCORES))],
                                ins=[a2a_in[g].opt()],
                                outs=[a2a_out[g].opt()],
                            )
                        # chunk g-1's collective finished while chunk g
                        # computed: its h-load won't block the sync queue
                        if g >= 1:
                            load_h_stripe(g - 1)
                    # ---- tail: block 1's out-proj split by K-tile parity.
                    # Pair-0's half-collective landed ~18us early, so the
                    # o2=0 K-tiles accumulate BEFORE the last collective;
                    # only the o2=1 pass + y writes remain after it. ----
                    load_h3(0)
                    # block 0's out-proj runs HERE, inside the last
                    # collective's latency window (its A2A chunks landed two
                    # chunks ago) — real work instead of idling, and 9us
                    # less PE work inside the stage-B span.  It must precede
                    # pass 1: pass 1's four py tiles hold ALL "s" PSUM bufs
                    # until pass 2, while block 0's recycle via its bias-adds.
                    stage_c_block(0)
                    it = 1
                    pys = {}
                    for hb in range(2):
                        for nch in range(2):
                            ns = slice(nch * 512, (nch + 1) * 512)
                            py = psS.tile([P, IB], F32, tag="s",
                                          name=f"py{hb}{nch}")
                            pys[hb, nch] = py
                            for s in range(GPB):
                                nc.tensor.matmul(
                                    py[:],
                                    h_sb[:, hb, s, 0, it * P:(it + 1) * P],
                                    wo_sb[:, 2 * s, ns],
                                    start=(s == 0),
                                    stop=False,
                                )
                    # a few dummies keep the HAM clock gate warm if the
                    # collective is still in flight
                    for w in range(8):
                        pw = psAV.tile([P, IB], F32, tag=f"av{w % 4}",
                                       name="pw")
                        nc.tensor.matmul(pw[:], identB[:], wo_sb[:, 0, 0:IB],
                                         start=True, stop=True)
                    load_h3(1)
                    for hb in range(2):
                        y_sb = ypool.tile([P, D], F32, tag="y")
                        for nch in range(2):
                            ns = slice(nch * 512, (nch + 1) * 512)
                            py = pys[hb, nch]
                            for s in range(GPB):
                                nc.tensor.matmul(
                                    py[:],
                                    h_sb[:, hb, s, 1, it * P:(it + 1) * P],
                                    wo_sb[:, 2 * s + 1, ns],
                                    start=False,
                                    stop=(s == GPB - 1),
                                )
                            nc.vector.tensor_tensor(
                                y_sb[:, ns], py[:], bo_full[:, ns], ALU.add)
                        r0 = hb * RB + it * P
                        for half in range(2):
                            eng = nc.sync if half == 0 else nc.scalar
                            eng.dma_start(
                                y[r0:r0 + P, half * 512:(half + 1) * 512],
                                y_sb[:, half * 512:(half + 1) * 512],
                            )

    nc.compile()
    return nc


_NC = None


def _get_nc():
    global _NC
    if _NC is None:
        _NC = build_nc()
    return _NC


def _make_in_maps(x, Wq, bq, Wk, bk, Wv, bv, Wo, bo):
    import ml_dtypes
    xf = np.asarray(x, np.float32).reshape(B, S, D)
    woT = np.ascontiguousarray(np.asarray(Wo, np.float32).T).astype(ml_dtypes.bfloat16)
    bo_r = np.ascontiguousarray(np.asarray(bo, np.float32).reshape(1, D))
    Wq = np.asarray(Wq, np.float32)
    Wk = np.asarray(Wk, np.float32)
    Wv = np.asarray(Wv, np.float32)
    in_maps = []
    for c in range(NCORES):
        b = c // GPB
        r = c % GPB
        sl = slice(r * CS, (r + 1) * CS)
        in_maps.append({
            "x": np.ascontiguousarray(xf[b]),
            "wqT": np.ascontiguousarray(Wq[sl, :].T).astype(ml_dtypes.bfloat16),
            "wkT": np.ascontiguousarray(Wk[sl, :].T).astype(ml_dtypes.bfloat16),
            "wvT": np.ascontiguousarray(Wv[sl, :].T).astype(ml_dtypes.bfloat16),
            "woT": woT,
            "bq": np.ascontiguousarray(
                np.asarray(bq, np.float32)[sl].reshape(NPAIR, P).T),
            "bk": np.ascontiguousarray(
                np.asarray(bk, np.float32)[sl].reshape(NPAIR, P).T),
            "bv": np.ascontiguousarray(
                np.asarray(bv, np.float32)[sl].reshape(NPAIR, P).T),
            "bo": bo_r,
        })
    return in_maps


def _assemble(results):
    yout = np.empty((B, S, D), np.float32)
    for d in range(NCORES):
        rows = slice(d * RB, (d + 1) * RB)
        yout[0, rows, :] = results[d]["y"][0:RB]
        yout[1, rows, :] = results[d]["y"][RB:2 * RB]
    return yout


def run_traced(trace=False, **inputs):
    """Run and return (output, BassKernelResults) — used by test.py."""
    nc = _get_nc()
    res = run_bass_kernel_spmd(
        nc, _make_in_maps(**inputs), core_ids=list(range(NCORES)), trace=trace
    )
    return _assemble(res.results), res


def kernel(**inputs) -> np.ndarray:
    out, _ = run_traced(trace=False, **inputs)
    return out
